# revision 27
# baseline (speedup 1.0000x reference)
"""DeepFM (embedding_lookup) Trainium2 Bass kernel.

Math: with idx[b,f,c] = sparse_feat[:, f*(C+1)+c] (c in [0,C)), all lookups of
field f hit the C-row table emb_tab[f].  Let count[b, f*C+v] = #{c : idx[b,f,c]==v}
(an exact integer histogram).  Then

  s[b,:]    = count[b] @ emb_flat           (emb_flat[f*C+v, :] = emb_tab[f,v,:])
  sqsum[b]  = count[b] @ rownorm            (rownorm[r] = |emb_flat[r]|^2)
  lin_wx[b] = count[b] @ (lin_emb*lin_W)    (per-field linear embeddings)
  out = sigmoid(0.5*(|s|^2 - sqsum) + lin_wx + dense@W_d + b)

On device (per core, batch-sharded 256 rows):
  - one-hot via 64 DVE tensor_scalar(is_equal) passes in bf16 (exact 0/1)
  - segmented reduce over c -> exact fp32 histogram
  - PE transposes + one K=1664 fp32 matmul against the on-device-assembled
    table T' = [emb | rownorm | lin_emb*lin_W]  -> [66, 256] per core
  - tiny PE epilogue (0.5|s|^2 - 0.5 sqsum + lin) + ACT sigmoid
"""

import numpy as np
import ml_dtypes
from contextlib import ExitStack

import concourse.bass as bass
import concourse.mybir as mybir
import concourse.tile as tile
from concourse import bacc
from concourse.bass_utils import run_bass_kernel_spmd
from concourse.masks import make_identity

B, F, C, E, D = 2048, 26, 64, 64, 13
NCORES = 8
BC = B // NCORES          # 256 batch rows per core
NBT = BC // 128           # 2 batch tiles of 128
R = F * C                 # 1664 table rows, f-major: r = f*C + v
NK = R // 128             # 13 K-chunks of 128 (2 fields per chunk)
VG = 4                    # histogram bins handled per one-hot buffer
FP32 = mybir.dt.float32
BF16 = mybir.dt.bfloat16

# staging column order: the four c-quarters of every field are split into four
# contiguous blocks [q][f][c%16] so on-device APs stay <=3 free dims and the
# first two c-sum tree levels can run as contiguous accumulate-DMAs
_COLS = np.array(
    [
        f * (C + 1) + q * (C // 4) + c
        for q in range(4)
        for f in range(F)
        for c in range(C // 4)
    ]
)

_BUILT = {}


def _emit(ctx, tc, idx_d, dense_d, emb_d, linemb_d, linw_d, linb_d, y_d):
    nc = tc.nc
    ts = bass.ts

    consts = ctx.enter_context(tc.tile_pool(name="consts", bufs=1))
    work = ctx.enter_context(tc.tile_pool(name="work", bufs=1))
    small = ctx.enter_context(tc.tile_pool(name="small", bufs=2))
    ohpool = ctx.enter_context(tc.tile_pool(name="ohpool", bufs=4))
    psum = ctx.enter_context(tc.tile_pool(name="psum", bufs=2, space="PSUM"))
    psum1 = ctx.enter_context(tc.tile_pool(name="psum1", bufs=1, space="PSUM"))

    # idx load first: everything else overlaps behind it
    idx_sb = work.tile([128, NBT, R], BF16)
    nc.sync.dma_start(
        out=idx_sb, in_=idx_d.ap().rearrange("(bt p) r -> p bt r", p=128)
    )

    # ---- constants ----
    ident = consts.tile([128, 128], FP32)
    make_identity(nc, ident)
    one_11 = consts.tile([1, 1], FP32)
    nc.gpsimd.memset(one_11, 1.0)
    half_col = consts.tile([64, 1], FP32)
    nc.gpsimd.memset(half_col, 0.5)
    ones2 = consts.tile([2, 1], FP32)
    nc.gpsimd.memset(ones2, 1.0)

    # ---- T' table assembly: tabT[p, kt, 0:64]=emb rows, 64=|row|^2, 65=lin ----
    tabT = work.tile([128, NK, 66], FP32)
    emb_rows = emb_d.ap().rearrange("f v e -> (f v) e").rearrange(
        "(kt p) e -> p kt e", p=128
    )
    nc.sync.dma_start(out=tabT[:, :, 0:64], in_=emb_rows)
    for kt in range(NK):
        sq_tmp = small.tile([128, 64], FP32)
        nc.scalar.activation(
            out=sq_tmp,
            in_=tabT[:, kt, 0:64],
            func=mybir.ActivationFunctionType.Square,
            accum_out=tabT[:, kt, 64:65],
        )
        # fold the -0.5 of the pairwise term into the |row|^2 column
        nc.scalar.mul(out=tabT[:, kt, 64:65], in_=tabT[:, kt, 64:65], mul=-0.5)

    lin_emb_flat = consts.tile([1, R], FP32)
    nc.sync.dma_start(
        out=lin_emb_flat, in_=linemb_d.ap().rearrange("(a f) v -> a (f v)", a=1)
    )
    linw_row = consts.tile([1, F], FP32)
    nc.sync.dma_start(
        out=linw_row,
        in_=linw_d.ap()[0:F, 0:1].rearrange("(a f) v -> a (f v)", a=1),
    )
    # scatter lin_emb[f*C+v]*lin_W[f] into tabT[:, kt, 65] (p = (f%2)*64+v)
    # via K=1 matmuls: out[64,1] = lin_emb[0:1, f*64:(f+1)*64].T @ [[lin_W[f]]]
    for kt in range(NK):
        p_lin = psum.tile([128, 1], FP32, tag="pscratch")
        for df in range(2):
            f = 2 * kt + df
            nc.tensor.matmul(
                p_lin[df * 64 : (df + 1) * 64, 0:1],
                lin_emb_flat[0:1, f * C : (f + 1) * C],
                linw_row[0:1, f : f + 1],
                start=True,
                stop=True,
            )
        nc.scalar.copy(out=tabT[:, kt, 65:66], in_=p_lin)

    # ---- dense/bias chunk: wdense [14, 66], densebias_t [14, 2, 128] ----
    wdense = consts.tile([14, 66], FP32)
    nc.gpsimd.memset(wdense, 0.0)
    nc.sync.dma_start(out=wdense[0:13, 65:66], in_=linw_d.ap()[26:39, 0:1])
    nc.sync.dma_start(out=wdense[13:14, 65:66], in_=linb_d.ap())

    densebias_t = work.tile([14, NBT, 128], FP32)
    nc.gpsimd.memset(densebias_t, 1.0)
    for bt in range(NBT):
        dense_sb = small.tile([128, D], FP32)
        nc.sync.dma_start(
            out=dense_sb, in_=dense_d.ap()[bt * 128 : (bt + 1) * 128, :]
        )
        p_d = psum.tile([D, 128], FP32, tag="pscratch")
        nc.tensor.transpose(p_d, dense_sb, ident)
        nc.scalar.copy(out=densebias_t[0:13, bt, :], in_=p_d)

    # ---- histogram: idx -> count[p, bt, (f,v)] (exact fp32 ints) ----
    count = work.tile([128, NBT, R], FP32)
    count_vbf = count.rearrange("p bt (f v) -> p v bt f", v=C)
    QTR = C // 4
    # idx arrives host-staged as [p, bt, q, f*QTR]: four c-quarter blocks
    idx_v = idx_sb.rearrange("p bt (q x) -> p bt q x", q=4)

    def tree(onehot, g, l1_on_dve=False, l2_on_dve=False):
        # remaining c-sum levels: in-place binary tree of bf16 adds on block 0
        # (2x DVE mode; tensor_reduce would run at 1x). Counts stay exact.
        flat = onehot.rearrange("p v bt q x -> p (v bt) q x")
        if l1_on_dve:
            pair = onehot.rearrange("p v bt q x -> p (v bt) (q x)")
            nc.vector.tensor_add(
                pair[:, :, 0 : 2 * F * QTR],
                pair[:, :, 0 : 2 * F * QTR],
                pair[:, :, 2 * F * QTR : 4 * F * QTR],
            )
        if l1_on_dve or l2_on_dve:
            nc.vector.tensor_add(flat[:, :, 0, :], flat[:, :, 0, :], flat[:, :, 1, :])
        blk = flat[:, :, 0, :].rearrange("p q (f c) -> p q f c", c=QTR)
        h = QTR // 2
        while h >= 1:
            a = blk[:, :, :, 0:h]
            b = blk[:, :, :, h : 2 * h]
            if h == 1:
                nc.vector.tensor_add(
                    count_vbf[:, g * VG : (g + 1) * VG, :, :],
                    a.rearrange("p q f c -> p q (f c)"),
                    b.rearrange("p q f c -> p q (f c)"),
                )
            else:
                nc.vector.tensor_add(a, a, b)
            h //= 2

    pending = []
    for g in range(C // VG):
        onehot = ohpool.tile([128, VG, NBT, 4, F * QTR], BF16)
        for j in range(VG):
            nc.vector.tensor_scalar(
                out=onehot[:, j, :, :, :],
                in0=idx_v,
                scalar1=float(g * VG + j),
                scalar2=None,
                op0=mybir.AluOpType.is_equal,
            )
        # c-sum level 1 (and level 2 on even groups) on the SWDGE accum-DMA
        # path: contiguous quarter-block adds, freeing the DVE for the rest.
        # The last group keeps everything on the DVE so the tail never waits.
        last = g == C // VG - 1
        l2_dma = (g % 2 == 0) and not last
        if not last:
            nc.gpsimd.dma_start(
                out=onehot[:, :, :, 0:2, :],
                in_=onehot[:, :, :, 2:4, :],
                accum_op=mybir.AluOpType.add,
            )
            if l2_dma:
                nc.gpsimd.dma_start(
                    out=onehot[:, :, :, 0, :],
                    in_=onehot[:, :, :, 1, :],
                    accum_op=mybir.AluOpType.add,
                )
        # emit trees two groups behind the compares so the in-order DVE queue
        # never stalls on the accum-DMAs
        pending.append((onehot, g, last, not l2_dma and not last))
        if len(pending) > 2:
            tree(*pending.pop(0))
    while pending:
        tree(*pending.pop(0))

    # ---- transpose count chunks -> count_t [128(f,v), bt, 128(b)] ----
    ct_all = work.tile([128, NK, NBT, 128], FP32)
    for kt in range(NK):
        for bt in range(NBT):
            p_t = psum.tile([128, 128], FP32)
            nc.tensor.transpose(p_t, count[:, bt, ts(kt, 128)], ident)
            nc.vector.tensor_copy(out=ct_all[:, kt, bt, :], in_=p_t)

    # ---- main matmul: y_t [66, bt, 128] = T'^T @ count_t (+ dense/bias) ----
    y_t = psum1.tile([66, NBT, 128], FP32)
    for kt in range(NK):
        nc.tensor.matmul(
            y_t,
            tabT[:, kt, :],
            ct_all[:, kt, :, :],
            start=(kt == 0),
            stop=False,
        )
    nc.tensor.matmul(y_t, wdense, densebias_t, start=False, stop=True)

    # ---- epilogue: z = 0.5|s|^2 - 0.5 sqsum + lin; out = sigmoid(z) ----
    for bt in range(NBT):
        s2_sb = small.tile([64, 128], FP32)
        nc.scalar.activation(
            out=s2_sb, in_=y_t[0:64, bt, :], func=mybir.ActivationFunctionType.Square
        )
        lin_sq = small.tile([2, 128], FP32)
        nc.scalar.copy(out=lin_sq, in_=y_t[64:66, bt, :])
        z_ps = psum.tile([128, 1], FP32, tag="pscratch")
        nc.tensor.matmul(z_ps, s2_sb, half_col, start=True, stop=False)
        nc.tensor.matmul(z_ps, lin_sq, ones2, start=False, stop=True)
        out_sb = small.tile([128, 1], FP32)
        nc.scalar.activation(
            out=out_sb, in_=z_ps, func=mybir.ActivationFunctionType.Sigmoid
        )
        nc.sync.dma_start(out=y_d.ap()[bt * 128 : (bt + 1) * 128, :], in_=out_sb)


def build():
    if "nc" in _BUILT:
        return _BUILT["nc"]
    nc = bacc.Bacc("TRN2", target_bir_lowering=False, debug=False)
    idx_d = nc.dram_tensor("idx", [BC, R], BF16, kind="ExternalInput")
    dense_d = nc.dram_tensor("dense", [BC, D], FP32, kind="ExternalInput")
    emb_d = nc.dram_tensor("emb", [F, C, E], FP32, kind="ExternalInput")
    linemb_d = nc.dram_tensor("linemb", [F, C], FP32, kind="ExternalInput")
    linw_d = nc.dram_tensor("linw", [F + D, 1], FP32, kind="ExternalInput")
    linb_d = nc.dram_tensor("linb", [1, 1], FP32, kind="ExternalInput")
    y_d = nc.dram_tensor("y", [BC, 1], FP32, kind="ExternalOutput")
    with tile.TileContext(nc) as tc:
        with ExitStack() as ctx:
            _emit(ctx, tc, idx_d, dense_d, emb_d, linemb_d, linw_d, linb_d, y_d)
    nc.compile()
    _BUILT["nc"] = nc
    return nc


def make_in_maps(sparse_feat, dense_feat, lin_emb, emb_tab, lin_W, lin_b):
    idx = np.asarray(sparse_feat)[:, _COLS].astype(ml_dtypes.bfloat16)
    dense = np.ascontiguousarray(np.asarray(dense_feat, dtype=np.float32))
    emb = np.ascontiguousarray(np.asarray(emb_tab, dtype=np.float32))
    linemb = np.ascontiguousarray(
        np.asarray(lin_emb, dtype=np.float32).reshape(F, C)
    )
    linw = np.ascontiguousarray(np.asarray(lin_W, dtype=np.float32).reshape(F + D, 1))
    linb = np.asarray(lin_b, dtype=np.float32).reshape(1, 1)
    in_maps = []
    for i in range(NCORES):
        sl = slice(i * BC, (i + 1) * BC)
        in_maps.append(
            {
                "idx": np.ascontiguousarray(idx[sl]),
                "dense": dense[sl],
                "emb": emb,
                "linemb": linemb,
                "linw": linw,
                "linb": linb,
            }
        )
    return in_maps


def kernel(sparse_feat, dense_feat, lin_emb, emb_tab, lin_W, lin_b):
    nc = build()
    in_maps = make_in_maps(sparse_feat, dense_feat, lin_emb, emb_tab, lin_W, lin_b)
    res = run_bass_kernel_spmd(nc, in_maps, list(range(NCORES)))
    return np.concatenate([r["y"] for r in res.results], axis=0)

